# revision 1
# baseline (speedup 1.0000x reference)
"""Bahdanau-attention kernel for Trainium2 (8 NeuronCores, Bass/Tile).

Computation (reference, fp32):
    Wh  = hidden @ W_w.T + W_b                      # [B, H]
    Ue  = einsum('bse,he->bsh', enc^T, U_w) + U_b   # [B, S, H]
    en  = tanh(Wh[:,None,:] + Ue) @ v_w[0]          # [B, S]
    out = softmax(where(mask, -1e10, en), axis=1)

Strategy
- Data-parallel over batch: 8 batches per core, weights replicated.
- Masked positions contribute exactly 0 to the softmax (exp(-1e10) = 0
  in fp32), so the host packs only the unmasked s-columns per batch row
  (padded to NP = max unmasked count rounded to a multiple of 4) and
  scatters results back; the device computes energies only for packed
  columns. This is exact, not an approximation. Fully-masked rows are
  uniform 1/S by definition and fixed up on the host.
- Main matmul out[h, s] = U_w.T-chunk (stationary) x enc-chunk (moving)
  in bf16 with fp32 PSUM accumulation; 16 k-chunks of 128 accumulate in
  one PSUM bank per (batch, h-chunk). Weights are host-rechunked per
  h-chunk so the first main block only needs 0.5 MB of weight DMA.
- Wh + W_b + U_b is folded into the tanh as a per-partition ACT bias;
  the Wh chains interleave with batch 0's main blocks so PE work paces
  the startup DMA stream (which is HBM-bandwidth-bound).
- The v-projection runs on the (otherwise idle) Vector engine: each
  tanh tile is scaled by its per-partition v chunk and accumulated over
  h-chunks in SBUF; a single ones-vector M=1 matmul per batch does the
  final partition reduce. This keeps the PE stream pure main-GEMM.
- Per-row softmax runs on partition 0 (no max-subtraction needed:
  |energy| < 32 so fp32 exp is safe; masked/padded columns give 0).

Host-side prep only reshapes/retypes/packs inputs; all FLOPs of the
module run on device in bf16/fp32.
"""

import numpy as np
import ml_dtypes

B, S, H, E = 64, 512, 1024, 2048
NCORES = 8
BL = B // NCORES          # batches per core
HC = H // 128             # h chunks
EC = E // 128             # e (contraction) chunks
KC = H // 128             # k chunks for the Wh matmul
NEG = np.float32(-1e10)

bf16 = ml_dtypes.bfloat16

_CACHE = {}


def _build_nc(NP):
    """Per-core program; NP = packed s-width (padded s-width, multiple of 4, <= 512)."""
    import concourse.mybir as mybir
    import concourse.tile as tile
    from concourse import bacc

    F32 = mybir.dt.float32
    BF = mybir.dt.bfloat16
    AF = mybir.ActivationFunctionType

    nc = bacc.Bacc(num_swdge_queues=4)
    enc_t = nc.declare_dram_parameter("enc_t", [E, BL, NP], BF, isOutput=False)
    # U_w.T pre-chunked by h-chunk: [hc, p(=e%128), ec, v(=h%128)], so the
    # first main block only needs the hc=0 slice (0.5 MB) instead of 4 MB
    uwT = nc.declare_dram_parameter("uwT", [HC, 128, EC, 128], BF, isOutput=False)
    # W_w.T re-chunked the same way: [hc, p(=k%128), kc, v(=h%128)]
    wwT = nc.declare_dram_parameter("wwT", [HC, 128, KC, 128], BF, isOutput=False)
    hidT = nc.declare_dram_parameter("hidT", [128, KC * BL], BF, isOutput=False)
    vt = nc.declare_dram_parameter("vt", [128, HC], BF, isOutput=False)
    bc = nc.declare_dram_parameter("bc", [128, HC], F32, isOutput=False)
    amask = nc.declare_dram_parameter("amask", [1, BL * NP], F32, isOutput=False)
    out_d = nc.declare_dram_parameter("out", [1, BL * NP], F32, isOutput=True)

    enc_r = enc_t.rearrange("(ec p) b s -> ec p b s", p=128)

    ENC_BUFS = 56

    with tile.TileContext(nc) as tc:
        with (
            tc.tile_pool(name="const", bufs=1) as cst,
            tc.tile_pool(name="wpool", bufs=1) as wp,
            tc.tile_pool(name="encp", bufs=ENC_BUFS) as encp,
            tc.tile_pool(name="thp", bufs=6) as thp,
            tc.tile_pool(name="accp", bufs=4) as accp,
            tc.tile_pool(name="smp", bufs=4) as smp,
            tc.tile_pool(name="pup", bufs=5, space="PSUM") as pup,
            tc.tile_pool(name="pep", bufs=2, space="PSUM") as pep,
            tc.tile_pool(name="pwp", bufs=1, space="PSUM") as pwp,
        ):
            # ---- constants / weights -------------------------------------
            # DMA order matters for the startup critical path: the Wh
            # prologue needs hid+ww first; the first main block needs uw.
            hid_sb = cst.tile([128, KC * BL], BF, tag="hid")
            nc.sync.dma_start(hid_sb[:], hidT[:])

            # per-hc weight chunks: each Wh chain / main block only needs its
            # own chunk, so PE work starts after a few hundred KB of DMA and
            # the rest streams in behind it. DMA order matches the b=0
            # interleave: (ww0, uw0, enc0) first, then (ww_k, uw_k) pairs.
            ww_sb = []
            for hc in range(HC):
                t = wp.tile([128, KC * 128], BF, tag=f"ww{hc}")
                ww_sb.append(t)
            uw_sb = []
            for hc in range(HC):
                t = wp.tile([128, EC * 128], BF, tag=f"uw{hc}")
                uw_sb.append(t)
            HALF = EC * 128 // 2
            KHALF = KC * 128 // 2

            nc.sync.dma_start(ww_sb[0][:, 0:KHALF], wwT[0, :, 0:KC // 2, :])
            nc.gpsimd.dma_start(ww_sb[0][:, KHALF:], wwT[0, :, KC // 2:, :])
            nc.sync.dma_start(uw_sb[0][:, 0:HALF], uwT[0, :, 0:EC // 2, :])
            nc.gpsimd.dma_start(uw_sb[0][:, HALF:], uwT[0, :, EC // 2:, :])
            bc_sb = cst.tile([128, HC], F32, tag="bc")
            nc.gpsimd.dma_start(bc_sb[:], bc[:])

            enc0_tiles = []
            for ec in range(EC):
                t = encp.tile([128, NP], BF, tag="enc")
                eng = (nc.sync, nc.gpsimd)[ec % 2]
                eng.dma_start(t[:], enc_r[ec, :, 0, :])
                enc0_tiles.append(t)

            for hc in range(1, HC):
                eng = (nc.sync, nc.gpsimd)[hc % 2]
                eng2 = (nc.gpsimd, nc.sync)[hc % 2]
                eng.dma_start(ww_sb[hc][:], wwT[hc])
                eng.dma_start(uw_sb[hc][:, 0:HALF], uwT[hc, :, 0:EC // 2, :])
                eng2.dma_start(uw_sb[hc][:, HALF:], uwT[hc, :, EC // 2:, :])
            vt_sb = cst.tile([128, HC], BF, tag="vt")
            nc.gpsimd.dma_start(vt_sb[:], vt[:])
            am_sb = cst.tile([1, BL * NP], F32, tag="am")
            nc.gpsimd.dma_start(am_sb[:], amask[:])

            bias_sb = cst.tile([128, HC * BL], F32, tag="bias")
            en_sb = cst.tile([1, BL * NP], F32, tag="en")
            res_sb = cst.tile([1, BL * NP], F32, tag="res")
            ones_sb = cst.tile([128, 1], BF, tag="ones")
            nc.vector.memset(ones_sb[:], 1.0)

            # ---- main loop over local batches ----------------------------
            # b=0 interleaves the Wh/bias prologue chain-by-chain with its
            # own main blocks so PE work paces with the weight DMA stream.
            for b in range(BL):
                if b == 0:
                    enc_tiles = enc0_tiles
                else:
                    enc_tiles = []
                    for ec in range(EC):
                        t = encp.tile([128, NP], BF, tag="enc")
                        eng = nc.sync if ec % 2 == 0 else nc.gpsimd
                        eng.dma_start(t[:], enc_r[ec, :, b, :])
                        enc_tiles.append(t)

                pe_ = pep.tile([1, NP], F32, tag="pe")
                acc = accp.tile([128, NP], F32, tag="acc")
                for hc in range(HC):
                    if b == 0:
                        # Wh chain for this h-chunk, feeding the tanh bias
                        pw = pwp.tile([128, BL], F32, tag="pw")
                        for kc in range(KC):
                            nc.tensor.matmul(
                                pw[:],
                                lhsT=ww_sb[hc][:, kc * 128:(kc + 1) * 128],
                                rhs=hid_sb[:, kc * BL:(kc + 1) * BL],
                                start=(kc == 0),
                                stop=(kc == KC - 1),
                            )
                        nc.vector.tensor_tensor(
                            bias_sb[:, hc * BL:(hc + 1) * BL], pw[:],
                            bc_sb[:, hc:hc + 1].to_broadcast([128, BL]),
                            mybir.AluOpType.add,
                        )
                    pu = pup.tile([128, NP], F32, tag="pu")
                    for ec in range(EC):
                        nc.tensor.matmul(
                            pu[:],
                            lhsT=uw_sb[hc][:, ec * 128:(ec + 1) * 128],
                            rhs=enc_tiles[ec][:],
                            start=(ec == 0),
                            stop=(ec == EC - 1),
                        )
                    th = thp.tile([128, NP], BF, tag="th")
                    nc.scalar.activation(
                        th[:], pu[:], AF.Tanh,
                        bias=bias_sb[:, hc * BL + b:hc * BL + b + 1],
                    )
                    # v-weighting on the (otherwise idle) Vector engine:
                    # acc[p, s] += v[hc*128+p] * tanh[p, s]
                    vcol = vt_sb[:, hc:hc + 1].to_broadcast([128, NP])
                    if hc == 0:
                        nc.vector.tensor_tensor(
                            acc[:], th[:], vcol, mybir.AluOpType.mult)
                    else:
                        tmp = thp.tile([128, NP], F32, tag="tmp")
                        nc.vector.tensor_tensor(
                            tmp[:], th[:], vcol, mybir.AluOpType.mult)
                        nc.vector.tensor_add(acc[:], acc[:], tmp[:])
                # single partition-reduce matmul replaces the 8 v-dots
                accb = thp.tile([128, NP], BF, tag="accb")
                nc.vector.tensor_copy(accb[:], acc[:])
                nc.tensor.matmul(
                    pe_[0:1, :], lhsT=ones_sb[:, 0:1], rhs=accb[:],
                    start=True, stop=True,
                )

                # ---- mask + softmax over packed columns on partition 0 ---
                sl = slice(b * NP, (b + 1) * NP)
                nc.vector.tensor_add(en_sb[0:1, sl], pe_[0:1, :], am_sb[0:1, sl])
                ssum = smp.tile([1, 1], F32, tag="ssum")
                nc.scalar.activation(
                    res_sb[0:1, sl], en_sb[0:1, sl], AF.Exp,
                    accum_out=ssum[0:1, 0:1],
                )
                rcp = smp.tile([1, 1], F32, tag="rcp")
                nc.vector.reciprocal(rcp[0:1, :], ssum[0:1, :])
                nc.vector.tensor_tensor(
                    res_sb[0:1, sl], res_sb[0:1, sl],
                    rcp[0:1, 0:1].to_broadcast([1, NP]),
                    mybir.AluOpType.mult,
                )
                nc.sync.dma_start(out_d[0:1, sl], res_sb[0:1, sl])

    nc.finalize()
    return nc


def _prep_inputs(hidden, encoder_outputs, mask, W_w, W_b, U_w, U_b, v_w):
    enc_bf = encoder_outputs.astype(bf16)          # [S, B, E]
    uwT_np = np.ascontiguousarray(U_w.T).astype(bf16)
    # re-chunk U_w.T [E, H] -> [hc, p, ec, v]: (e=ec*128+p, h=hc*128+v)
    uwT_np = np.ascontiguousarray(
        uwT_np.reshape(EC, 128, HC, 128).transpose(2, 1, 0, 3))
    wwT_np = np.ascontiguousarray(W_w.T).astype(bf16)
    wwT_np = np.ascontiguousarray(
        wwT_np.reshape(KC, 128, HC, 128).transpose(2, 1, 0, 3))
    vt_np = np.ascontiguousarray(v_w[0].reshape(HC, 128).T).astype(bf16)
    bc_np = np.ascontiguousarray((W_b + U_b).reshape(HC, 128).T).astype(np.float32)

    idx_all = [np.nonzero(~mask[i])[0] for i in range(B)]
    counts = np.array([len(ix) for ix in idx_all])
    NP = int(max(64, 4 * -(-counts.max() // 4)))  # ceil to multiple of 4

    in_maps = []
    for c in range(NCORES):
        bsl = slice(c * BL, (c + 1) * BL)
        enc_c = np.ascontiguousarray(enc_bf[:, bsl, :].transpose(2, 1, 0))  # [E, BL, S]
        enc_p = np.zeros((E, BL, NP), bf16)
        am_p = np.full((BL, NP), NEG, np.float32)
        for b in range(BL):
            ix = idx_all[c * BL + b]
            cnt = len(ix)
            if cnt:
                enc_p[:, b, :cnt] = enc_c[:, b, ix]
                am_p[b, :cnt] = 0.0
        hid_c = hidden[bsl].astype(bf16)                                    # [BL, H]
        hidT_c = np.ascontiguousarray(
            hid_c.T.reshape(KC, 128, BL).transpose(1, 0, 2)
        ).reshape(128, KC * BL)
        in_maps.append({
            "enc_t": enc_p,
            "uwT": uwT_np,
            "wwT": wwT_np,
            "hidT": hidT_c,
            "vt": vt_np,
            "bc": bc_np,
            "amask": am_p.reshape(1, BL * NP),
        })
    return in_maps, NP, idx_all, counts


def _run(in_maps, NP, trace=False):
    from concourse import bass_utils
    if NP not in _CACHE:
        _CACHE[NP] = _build_nc(NP)
    nc = _CACHE[NP]
    return bass_utils.run_bass_kernel_spmd(
        nc, in_maps, core_ids=list(range(NCORES)), trace=trace
    )


def kernel(hidden, encoder_outputs, mask, W_w, W_b, U_w, U_b, v_w,
           _trace=False, _return_bkr=False):
    hidden = np.asarray(hidden, dtype=np.float32)
    encoder_outputs = np.asarray(encoder_outputs, dtype=np.float32)
    mask = np.asarray(mask).astype(bool)
    W_w = np.asarray(W_w, dtype=np.float32)
    W_b = np.asarray(W_b, dtype=np.float32)
    U_w = np.asarray(U_w, dtype=np.float32)
    U_b = np.asarray(U_b, dtype=np.float32)
    v_w = np.asarray(v_w, dtype=np.float32)

    in_maps, NP, idx_all, counts = _prep_inputs(
        hidden, encoder_outputs, mask, W_w, W_b, U_w, U_b, v_w)
    bkr = _run(in_maps, NP, trace=_trace)

    out = np.zeros((B, S), np.float32)
    for c in range(NCORES):
        dev = bkr.results[c]["out"].reshape(BL, NP)
        for b in range(BL):
            i = c * BL + b
            cnt = counts[i]
            if cnt:
                out[i, idx_all[i]] = dev[b, :cnt]
            else:
                # fully-masked row: softmax over all -1e10 is uniform
                out[i, :] = np.float32(1.0 / S)
    if _return_bkr:
        return out, bkr
    return out



# revision 7
# speedup vs baseline: 1.2832x; 1.2832x over previous
"""Bahdanau-attention kernel for Trainium2 (8 NeuronCores, Bass/Tile).

Computation (reference, fp32):
    Wh  = hidden @ W_w.T + W_b                      # [B, H]
    Ue  = einsum('bse,he->bsh', enc^T, U_w) + U_b   # [B, S, H]
    en  = tanh(Wh[:,None,:] + Ue) @ v_w[0]          # [B, S]
    out = softmax(where(mask, -1e10, en), axis=1)

Strategy (v2 - fp8 DoubleRow):
- Data-parallel over batch: 8 rows per core. Rows are rank-sorted by
  unmasked count so position p on every core is padded to the same
  compile-time width w[p]; only unmasked s-columns are packed (exact:
  exp(-1e10) = 0).
- Main GEMM in fp8 e4m3 with perf_mode=DoubleRow: each matmul contracts
  256 rows (two 128-chunks), halving PE streaming time vs bf16. U is
  scaled by 64 before quantization; the tanh activation un-scales.
- Positions are bin-packed into column tiles <= 512 (one PSUM bank
  each). Loop order per h-chunk is ec2-major: one weight pair feeds all
  bins back-to-back so LDWEIGHTS overlaps the matmul stream.
- Wh (bf16) runs as small interleaved chains; W_b+U_b folded into the
  per-partition tanh bias.
- v-weighting runs on the Vector engine (tmp = tanh*v; acc += tmp, fp32;
  the last h-chunk's add writes a bf16 copy); final partition reduce is
  one bf16 ones-matmul per position; the mask lands in PSUM via a K=1
  bf16 matmul of the -1e10 row.
- Per-position softmax on PSUM strips (partition 32*(i%4) of 2 energy
  banks): Exp with accum_out, reciprocal, one broadcast multiply per
  group of 4 positions.
"""

import numpy as np
import ml_dtypes

B, S, H, E = 64, 512, 1024, 2048
NCORES = 8
BL = B // NCORES          # rows (positions) per core
HC = H // 128             # h chunks
EC2 = E // 256            # fp8 DoubleRow chunk pairs
KC = H // 128             # k chunks for the Wh matmul
NEG = np.float32(-1e10)
U_SCALE = 64.0

bf16 = ml_dtypes.bfloat16
fp8 = ml_dtypes.float8_e4m3

_CACHE = {}


def _plan(widths):
    """Bin-pack position widths into column tiles <= 512 (first-fit dec.).

    Returns (placements, bins) where placements is a list of original
    position indices in column order and bins is a list of
    (bin_off, bin_w, [(placement_idx, local_off, w), ...]).
    """
    order = sorted(range(len(widths)), key=lambda p: -widths[p])
    bins = []  # list of lists of original position idx
    for p in order:
        for bn in bins:
            if sum(widths[q] for q in bn) + widths[p] <= 512:
                bn.append(p)
                break
        else:
            bins.append([p])
    placements = [p for bn in bins for p in bn]
    out_bins = []
    off = 0
    pi = 0
    for bn in bins:
        lo = 0
        segs = []
        for p in bn:
            segs.append((pi, lo, widths[p]))
            lo += widths[p]
            pi += 1
        out_bins.append((off, lo, segs))
        off += lo
    return placements, out_bins


def _build_nc(widths):
    """Per-core program; widths = per-position packed col counts (<=512)."""
    import concourse.mybir as mybir
    import concourse.tile as tile
    from concourse import bacc

    F32 = mybir.dt.float32
    F32R = mybir.dt.float32r
    BF = mybir.dt.bfloat16
    F8 = mybir.dt.float8e4
    AF = mybir.ActivationFunctionType
    DR = mybir.MatmulPerfMode.DoubleRow

    placements, bins = _plan(widths)
    NPOS = len(widths)
    X = sum(widths)
    XP = -(-X // 16) * 16           # plane stride must be 16B-aligned
    NB = len(bins)
    assert NB + 3 <= 8, f"too many PSUM banks: {NB}"
    # placement -> (col_off, width)
    pl_off = [None] * NPOS
    for boff, bw, segs in bins:
        for pi, lo, w in segs:
            pl_off[pi] = (boff + lo, w)

    nc = bacc.Bacc(num_swdge_queues=4)
    enc8 = nc.declare_dram_parameter("enc8", [EC2, 128, 2 * XP], F8, isOutput=False)
    uw8 = nc.declare_dram_parameter("uw8", [HC, 128, EC2 * 2 * 128], F8, isOutput=False)
    wwT = nc.declare_dram_parameter("wwT", [HC, 128, KC * 128], BF, isOutput=False)
    hidT = nc.declare_dram_parameter("hidT", [128, KC * BL], BF, isOutput=False)
    vt = nc.declare_dram_parameter("vt", [128, HC], BF, isOutput=False)
    bc = nc.declare_dram_parameter("bc", [128, HC], F32, isOutput=False)
    amask = nc.declare_dram_parameter("amask", [1, XP], BF, isOutput=False)
    out_d = nc.declare_dram_parameter("out", [NPOS, 512], F32, isOutput=True)

    with tile.TileContext(nc) as tc:
        with (
            tc.tile_pool(name="const", bufs=1) as cst,
            tc.tile_pool(name="wpool", bufs=1) as wp,
            tc.tile_pool(name="encp", bufs=EC2) as encp,
            tc.tile_pool(name="thp", bufs=4) as thp,
            tc.tile_pool(name="tmpp", bufs=3) as tmpp,
            tc.tile_pool(name="pwp", bufs=1, space="PSUM") as pwp,
            tc.tile_pool(name="pup", bufs=NB, space="PSUM") as pup,
            tc.tile_pool(name="pep", bufs=1, space="PSUM") as pep,
        ):
            # ---- DMAs, roughly in the order the PE stream consumes ----
            hid_sb = cst.tile([128, KC * BL], BF, tag="hid")
            nc.sync.dma_start(hid_sb[:], hidT[:])
            bc_sb = cst.tile([128, HC], F32, tag="bc")
            nc.gpsimd.dma_start(bc_sb[:], bc[:])
            vt_sb = cst.tile([128, HC], BF, tag="vt")
            nc.gpsimd.dma_start(vt_sb[:], vt[:])
            am_sb = cst.tile([1, XP], BF, tag="am")
            nc.gpsimd.dma_start(am_sb[:], amask[:])

            ww_sb = [wp.tile([128, KC * 128], BF, tag=f"ww{h}", name=f"ww{h}") for h in range(HC)]
            uw_sb = [wp.tile([128, EC2 * 256], F8, tag=f"uw{h}", name=f"uw{h}") for h in range(HC)]
            enc_sb = [encp.tile([128, 2 * XP], F8, tag="enc", name=f"enc{k}") for k in range(EC2)]

            # startup-critical: ww0, uw0, enc0 first; then ww_k / enc_k
            # interleaved; remaining uw chunks trail.
            def dma2(dst, src, width):
                h = (width // 2) & ~3
                nc.sync.dma_start(dst[:, 0:h], src[:, 0:h])
                nc.gpsimd.dma_start(dst[:, h:width], src[:, h:width])

            dma2(ww_sb[0], wwT[0], KC * 128)
            dma2(uw_sb[0], uw8[0], EC2 * 256)
            dma2(enc_sb[0], enc8[0], 2 * XP)
            for k in range(1, EC2):
                dma2(ww_sb[k], wwT[k], KC * 128)
                if k >= 5:
                    dma2(uw_sb[k - 4], uw8[k - 4], EC2 * 256)
                dma2(enc_sb[k], enc8[k], 2 * XP)
            for h in range(4, HC):
                dma2(uw_sb[h], uw8[h], EC2 * 256)

            bias_sb = cst.tile([128, HC * BL], F32, tag="bias")
            acc = cst.tile([128, XP], F32, tag="acc")
            accb = cst.tile([128, XP], BF, tag="accb")
            ones_c = cst.tile([128, 1], BF, tag="onesc")
            nc.vector.memset(ones_c[:], 1.0)
            ones_bf = cst.tile([1, 1], BF, tag="onesbf")
            nc.vector.memset(ones_bf[:], 1.0)

            NG = (NPOS + 3) // 4
            res_g = [cst.tile([128, 512], F32, tag=f"res{g}", name=f"res{g}") for g in range(NG)]
            ss_g = [cst.tile([128, 1], F32, tag=f"ss{g}", name=f"ss{g}") for g in range(NG)]
            rcp_g = [cst.tile([128, 1], F32, tag=f"rcp{g}", name=f"rcp{g}") for g in range(NG)]
            for g in range(NG):
                nc.vector.memset(res_g[g][:], 0.0)
                nc.vector.memset(ss_g[g][:], 1.0)

            # ---- main loop over h-chunks ------------------------------
            for hc in range(HC):
                # Wh chain for this h-chunk (bf16), bias = pw + (W_b+U_b)
                pw = pwp.tile([128, BL], F32, tag="pw")
                for kc in range(KC):
                    nc.tensor.matmul(
                        pw[:],
                        lhsT=ww_sb[hc][:, kc * 128:(kc + 1) * 128],
                        rhs=hid_sb[:, kc * BL:(kc + 1) * BL],
                        start=(kc == 0),
                        stop=(kc == KC - 1),
                    )
                nc.vector.tensor_tensor(
                    bias_sb[:, hc * BL:(hc + 1) * BL], pw[:],
                    bc_sb[:, hc:hc + 1].to_broadcast([128, BL]),
                    mybir.AluOpType.add,
                )

                # main fp8 DoubleRow GEMM, ec2-major so each weight pair
                # feeds all bins back-to-back
                uw3 = uw_sb[hc][:].rearrange("p (e two v) -> p e two v", two=2, v=128)
                psb = [pup.tile([128, bw], F32, tag="pu", name=f"pu{t}") for t, (_, bw, _) in enumerate(bins)]
                for ec2 in range(EC2):
                    w3 = uw3[:, ec2]
                    e3 = enc_sb[ec2][:].rearrange("p (two x) -> p two x", two=2)
                    for t, (boff, bw, _) in enumerate(bins):
                        nc.tensor.matmul(
                            psb[t][:],
                            lhsT=w3,
                            rhs=e3[:, :, boff:boff + bw],
                            start=(ec2 == 0),
                            stop=(ec2 == EC2 - 1),
                            perf_mode=DR,
                        )
                # tanh (+ per-position Wh bias), then v-weight into acc
                vcol = vt_sb[:, hc:hc + 1]
                for t, (boff, bw, segs) in enumerate(bins):
                    th = thp.tile([128, bw], BF, tag="th")
                    for pi, lo, w in segs:
                        # hidT columns are stored in placement order, so the
                        # Wh bias column for this segment is pi itself
                        nc.scalar.activation(
                            th[:, lo:lo + w], psb[t][:, lo:lo + w], AF.Tanh,
                            bias=bias_sb[:, hc * BL + pi:hc * BL + pi + 1],
                            scale=1.0 / U_SCALE,
                        )
                    if hc == 0:
                        nc.vector.tensor_tensor(
                            acc[:, boff:boff + bw], th[:],
                            vcol.to_broadcast([128, bw]), mybir.AluOpType.mult)
                    else:
                        tmp = tmpp.tile([128, bw], F32, tag="tmp")
                        nc.vector.tensor_tensor(
                            tmp[:], th[:], vcol.to_broadcast([128, bw]),
                            mybir.AluOpType.mult)
                        # last h-chunk: write the sum as bf16 for the
                        # partition-reduce matmul (single final rounding)
                        dst = accb if hc == HC - 1 else acc
                        nc.vector.tensor_add(
                            dst[:, boff:boff + bw], acc[:, boff:boff + bw], tmp[:])

            # ---- partition reduce + mask + softmax per position -------
            pe_g = [pep.tile([128, 512], F32, tag=f"pe{g}", name=f"pe{g}") for g in range(NG)]
            for pi in range(NPOS):
                off, w = pl_off[pi]
                g, strip = pi // 4, 32 * (pi % 4)
                pslice = pe_g[g][strip:strip + 1, 0:w]
                nc.tensor.matmul(
                    pslice, lhsT=ones_bf[0:1, 0:1],
                    rhs=am_sb[0:1, off:off + w],
                    start=True, stop=False, tile_position=(0, strip),
                )
                nc.tensor.matmul(
                    pslice, lhsT=ones_c[:, 0:1],
                    rhs=accb[:, off:off + w],
                    start=False, stop=True, tile_position=(0, strip),
                )
                nc.scalar.activation(
                    res_g[g][strip:strip + 1, 0:w],
                    pe_g[g][strip:strip + 1, 0:w], AF.Exp,
                    accum_out=ss_g[g][strip:strip + 1, 0:1],
                )
            for g in range(NG):
                nc.vector.reciprocal(rcp_g[g][:], ss_g[g][:])
                nc.vector.tensor_tensor(
                    res_g[g][:], res_g[g][:],
                    rcp_g[g][:, 0:1].to_broadcast([128, 512]),
                    mybir.AluOpType.mult,
                )
            for pi in range(NPOS):
                off, w = pl_off[pi]
                g, strip = pi // 4, 32 * (pi % 4)
                eng = (nc.sync, nc.gpsimd)[pi % 2]
                eng.dma_start(out_d[pi:pi + 1, 0:w], res_g[g][strip:strip + 1, 0:w])

    nc.finalize()
    return nc


def _prep_inputs(hidden, encoder_outputs, mask, W_w, W_b, U_w, U_b, v_w):
    counts = (~mask).sum(axis=1)
    order = np.argsort(counts, kind="stable")
    # position p on core c holds row order[NCORES*p + c]
    rows = order.reshape(BL, NCORES)
    widths = [max(16, int(counts[rows[p]].max())) for p in range(BL)]
    widths = tuple(widths)
    assert all(w <= 512 for w in widths)

    placements, bins = _plan(widths)
    X = sum(widths)
    XP = -(-X // 16) * 16
    pl_off = [None] * BL
    for boff, bw, segs in bins:
        for pi, lo, w in segs:
            pl_off[pi] = (boff + lo, w)
    # original position -> placement idx
    pos2pl = [None] * BL
    for pi, p in enumerate(placements):
        pos2pl[p] = pi

    # ---- replicated weights ----
    U8 = np.ascontiguousarray((U_w * U_SCALE).T).astype(fp8)       # [E, H]
    uw8_np = np.ascontiguousarray(
        U8.reshape(EC2, 2, 128, HC, 128).transpose(3, 2, 0, 1, 4)
    ).reshape(HC, 128, EC2 * 2 * 128)
    wwT_np = np.ascontiguousarray(W_w.T).astype(bf16)
    wwT_np = np.ascontiguousarray(
        wwT_np.reshape(KC, 128, HC, 128).transpose(2, 1, 0, 3)
    ).reshape(HC, 128, KC * 128)
    vt_np = np.ascontiguousarray(v_w[0].reshape(HC, 128).T).astype(bf16)
    bc_np = np.ascontiguousarray((W_b + U_b).reshape(HC, 128).T).astype(np.float32)

    enc8_full = encoder_outputs.astype(fp8)                        # [S, B, E]

    idx_all = [np.nonzero(~mask[i])[0] for i in range(B)]
    in_maps = []
    for c in range(NCORES):
        enc_p = np.zeros((EC2, 128, 2, XP), fp8)
        am_p = np.full((XP,), NEG, np.float32)
        hid_rows = np.empty((BL, H), np.float32)
        for p in range(BL):
            r = int(rows[p, c])
            pi = pos2pl[p]
            off, w = pl_off[pi]
            ix = idx_all[r]
            cnt = len(ix)
            if cnt:
                # [cnt, E] -> [EC2, 2, 128, cnt]
                a = enc8_full[ix, r, :].T.reshape(EC2, 2, 128, cnt)
                enc_p[:, :, :, off:off + cnt] = a.transpose(0, 2, 1, 3)
                am_p[off:off + cnt] = 0.0
            hid_rows[pi] = hidden[r]
        hidT_c = np.ascontiguousarray(
            hid_rows.T.astype(bf16).reshape(KC, 128, BL).transpose(1, 0, 2)
        ).reshape(128, KC * BL)
        in_maps.append({
            "enc8": enc_p.reshape(EC2, 128, 2 * XP),
            "uw8": uw8_np,
            "wwT": wwT_np,
            "hidT": hidT_c,
            "vt": vt_np,
            "bc": bc_np,
            "amask": am_p.astype(bf16).reshape(1, XP),
        })
    return in_maps, widths, rows, pos2pl, idx_all


def _run(in_maps, widths, trace=False):
    from concourse import bass_utils
    if widths not in _CACHE:
        _CACHE[widths] = _build_nc(widths)
    nc = _CACHE[widths]
    return bass_utils.run_bass_kernel_spmd(
        nc, in_maps, core_ids=list(range(NCORES)), trace=trace
    )


def kernel(hidden, encoder_outputs, mask, W_w, W_b, U_w, U_b, v_w,
           _trace=False, _return_bkr=False):
    hidden = np.asarray(hidden, dtype=np.float32)
    encoder_outputs = np.asarray(encoder_outputs, dtype=np.float32)
    mask = np.asarray(mask).astype(bool)
    W_w = np.asarray(W_w, dtype=np.float32)
    W_b = np.asarray(W_b, dtype=np.float32)
    U_w = np.asarray(U_w, dtype=np.float32)
    U_b = np.asarray(U_b, dtype=np.float32)
    v_w = np.asarray(v_w, dtype=np.float32)

    in_maps, widths, rows, pos2pl, idx_all = _prep_inputs(
        hidden, encoder_outputs, mask, W_w, W_b, U_w, U_b, v_w)
    bkr = _run(in_maps, widths, trace=_trace)

    out = np.zeros((B, S), np.float32)
    for c in range(NCORES):
        dev = bkr.results[c]["out"]                  # [BL, 512]
        for p in range(BL):
            r = int(rows[p, c])
            ix = idx_all[r]
            cnt = len(ix)
            if cnt:
                out[r, ix] = dev[pos2pl[p], :cnt]
            else:
                out[r, :] = np.float32(1.0 / S)
    if _return_bkr:
        return out, bkr
    return out


# revision 12
# speedup vs baseline: 1.5765x; 1.2285x over previous
"""Bahdanau-attention kernel for Trainium2 (8 NeuronCores, Bass/Tile).

Computation (reference, fp32):
    Wh  = hidden @ W_w.T + W_b                      # [B, H]
    Ue  = einsum('bse,he->bsh', enc^T, U_w) + U_b   # [B, S, H]
    en  = tanh(Wh[:,None,:] + Ue) @ v_w[0]          # [B, S]
    out = softmax(where(mask, -1e10, en), axis=1)

Strategy (v2 - fp8 DoubleRow):
- Data-parallel over batch: 8 rows per core. Rows are rank-sorted by
  unmasked count so position p on every core is padded to the same
  compile-time width w[p]; only unmasked s-columns are packed (exact:
  exp(-1e10) = 0).
- Main GEMM in fp8 e4m3 with perf_mode=DoubleRow: each matmul contracts
  256 rows (two 128-chunks), halving PE streaming time vs bf16. U is
  scaled by 64 before quantization; the tanh activation un-scales.
- Positions are bin-packed into column tiles <= 512 (one PSUM bank
  each). Loop order per h-chunk is ec2-major: one weight pair feeds all
  bins back-to-back so LDWEIGHTS overlaps the matmul stream.
- Wh (bf16) runs as small interleaved chains; W_b+U_b folded into the
  per-partition tanh bias.
- v-weighting runs on the Vector engine (tmp = tanh*v in bf16; acc +=
  tmp in fp32; the last h-chunk's add writes a bf16 copy and the -1e10
  mask row is added into partition 127); final partition reduce is one
  bf16 ones-matmul per position into PSUM strips (partition 32*(i%4)).
- Softmax: one bank-wide Exp (+per-partition accum) per group of 4
  positions, reciprocal, one broadcast multiply, one 4-row output DMA.
- DMA: both HWDGE queues (sync + scalar) stream enc/uw with e0/uw0
  first; gpsimd SWDGE carries the later-needed ww/uw chunks.
"""

import numpy as np
import ml_dtypes

B, S, H, E = 64, 512, 1024, 2048
NCORES = 8
BL = B // NCORES          # rows (positions) per core
HC = H // 128             # h chunks
EC2 = E // 256            # fp8 DoubleRow chunk pairs
KC = H // 128             # k chunks for the Wh matmul
NEG = np.float32(-1e10)
U_SCALE = 64.0

bf16 = ml_dtypes.bfloat16
fp8 = ml_dtypes.float8_e4m3

_CACHE = {}


def _plan(widths):
    """Bin-pack position widths into column tiles <= 512 (first-fit dec.).

    Returns (placements, bins) where placements is a list of original
    position indices in column order and bins is a list of
    (bin_off, bin_w, [(placement_idx, local_off, w), ...]).
    """
    order = sorted(range(len(widths)), key=lambda p: -widths[p])
    bins = []  # list of lists of original position idx
    for p in order:
        for bn in bins:
            if sum(widths[q] for q in bn) + widths[p] <= 512:
                bn.append(p)
                break
        else:
            bins.append([p])
    placements = [p for bn in bins for p in bn]
    out_bins = []
    off = 0
    pi = 0
    for bn in bins:
        lo = 0
        segs = []
        for p in bn:
            segs.append((pi, lo, widths[p]))
            lo += widths[p]
            pi += 1
        out_bins.append((off, lo, segs))
        off += lo
    return placements, out_bins


def _build_nc(widths):
    """Per-core program; widths = per-position packed col counts (<=512)."""
    import concourse.mybir as mybir
    import concourse.tile as tile
    from concourse import bacc

    F32 = mybir.dt.float32
    F32R = mybir.dt.float32r
    BF = mybir.dt.bfloat16
    F8 = mybir.dt.float8e4
    AF = mybir.ActivationFunctionType
    DR = mybir.MatmulPerfMode.DoubleRow

    placements, bins = _plan(widths)
    NPOS = len(widths)
    X = sum(widths)
    XP = -(-X // 16) * 16           # plane stride must be 16B-aligned
    NB = len(bins)
    assert NB + 3 <= 8, f"too many PSUM banks: {NB}"
    # placement -> (col_off, width)
    pl_off = [None] * NPOS
    for boff, bw, segs in bins:
        for pi, lo, w in segs:
            pl_off[pi] = (boff + lo, w)

    nc = bacc.Bacc(num_swdge_queues=4)
    enc8 = nc.declare_dram_parameter("enc8", [EC2, 128, 2 * XP], F8, isOutput=False)
    uw8 = nc.declare_dram_parameter("uw8", [HC, 128, EC2 * 2 * 128], F8, isOutput=False)
    wwT = nc.declare_dram_parameter("wwT", [HC, 128, KC * 128], BF, isOutput=False)
    hidT = nc.declare_dram_parameter("hidT", [128, KC * BL], BF, isOutput=False)
    vt = nc.declare_dram_parameter("vt", [128, HC], BF, isOutput=False)
    bc = nc.declare_dram_parameter("bc", [128, HC], F32, isOutput=False)
    amask = nc.declare_dram_parameter("amask", [1, XP], BF, isOutput=False)
    NG = (NPOS + 3) // 4
    corr = nc.declare_dram_parameter("corr", [128, NG], F32, isOutput=False)
    out_d = nc.declare_dram_parameter("out", [NPOS, 512], F32, isOutput=True)

    with tile.TileContext(nc) as tc:
        with (
            tc.tile_pool(name="const", bufs=1) as cst,
            tc.tile_pool(name="wpool", bufs=1) as wp,
            tc.tile_pool(name="encp", bufs=EC2) as encp,
            tc.tile_pool(name="thp", bufs=4) as thp,
            tc.tile_pool(name="tmpp", bufs=3) as tmpp,
            tc.tile_pool(name="pwp", bufs=1, space="PSUM") as pwp,
            tc.tile_pool(name="pup", bufs=NB, space="PSUM") as pup,
            tc.tile_pool(name="pep", bufs=1, space="PSUM") as pep,
        ):
            # ---- DMAs: sync + scalar are HWDGE queues (fast); gpsimd is
            # software DGE and carries only later-needed weights/constants.
            hid_sb = cst.tile([128, KC * BL], BF, tag="hid")
            nc.sync.dma_start(hid_sb[:], hidT[:])
            bc_sb = cst.tile([128, HC], F32, tag="bc")
            nc.gpsimd.dma_start(bc_sb[:], bc[:])
            vt_sb = cst.tile([128, HC], BF, tag="vt")
            nc.gpsimd.dma_start(vt_sb[:], vt[:])
            am_sb = cst.tile([1, XP], BF, tag="am")
            nc.gpsimd.dma_start(am_sb[:], amask[:])

            ww_sb = [wp.tile([128, KC * 128], BF, tag=f"ww{h}", name=f"ww{h}") for h in range(HC)]
            uw_sb = [wp.tile([128, EC2 * 256], F8, tag=f"uw{h}", name=f"uw{h}") for h in range(HC)]
            enc_sb = [encp.tile([128, 2 * XP], F8, tag="enc", name=f"enc{k}") for k in range(EC2)]

            def dma2(dst, src, width, e1, e2):
                h = (width // 2) & ~3
                e1.dma_start(dst[:, 0:h], src[:, 0:h])
                e2.dma_start(dst[:, h:width], src[:, h:width])

            # startup-critical, split across both HWDGE queues
            dma2(ww_sb[0], wwT[0], KC * 128, nc.sync, nc.scalar)
            dma2(uw_sb[0], uw8[0], EC2 * 256, nc.sync, nc.scalar)
            dma2(enc_sb[0], enc8[0], 2 * XP, nc.sync, nc.scalar)
            for k in range(1, EC2):
                dma2(enc_sb[k], enc8[k], 2 * XP, nc.sync, nc.scalar)
                if k == 2:
                    nc.scalar.dma_start(uw_sb[1][:], uw8[1][:])
                    nc.sync.dma_start(uw_sb[2][:], uw8[2][:])
                if k == 5:
                    nc.scalar.dma_start(uw_sb[3][:], uw8[3][:])
            # Wh weights k>=1 and uw k>=4 arrive later: gpsimd SWDGE,
            # in the order the PE stream consumes them
            for k in range(1, HC):
                nc.gpsimd.dma_start(ww_sb[k][:], wwT[k][:])
            for h in range(4, HC):
                nc.gpsimd.dma_start(uw_sb[h][:], uw8[h][:])

            bias_sb = cst.tile([128, HC * BL], F32, tag="bias")
            acc = cst.tile([128, XP], F32, tag="acc")
            accb = cst.tile([128, XP], BF, tag="accb")
            ones_c = cst.tile([128, 1], BF, tag="onesc")
            nc.vector.memset(ones_c[:], 1.0)

            corr_sb = cst.tile([128, NG], F32, tag="corr")
            nc.gpsimd.dma_start(corr_sb[:], corr[:])
            res_g = [cst.tile([128, 512], F32, tag=f"res{g}", name=f"res{g}") for g in range(NG)]
            ss_g = [cst.tile([128, 1], F32, tag=f"ss{g}", name=f"ss{g}") for g in range(NG)]
            rcp_g = [cst.tile([128, 1], F32, tag=f"rcp{g}", name=f"rcp{g}") for g in range(NG)]
            # ---- main loop over h-chunks ------------------------------
            for hc in range(HC):
                # Wh chain for this h-chunk (bf16), bias = pw + (W_b+U_b)
                pw = pwp.tile([128, BL], F32, tag="pw")
                for kc in range(KC):
                    nc.tensor.matmul(
                        pw[:],
                        lhsT=ww_sb[hc][:, kc * 128:(kc + 1) * 128],
                        rhs=hid_sb[:, kc * BL:(kc + 1) * BL],
                        start=(kc == 0),
                        stop=(kc == KC - 1),
                    )
                nc.vector.tensor_tensor(
                    bias_sb[:, hc * BL:(hc + 1) * BL], pw[:],
                    bc_sb[:, hc:hc + 1].to_broadcast([128, BL]),
                    mybir.AluOpType.add,
                )

                # main fp8 DoubleRow GEMM, ec2-major so each weight pair
                # feeds all bins back-to-back
                uw3 = uw_sb[hc][:].rearrange("p (e two v) -> p e two v", two=2, v=128)
                psb = [pup.tile([128, bw], F32, tag="pu", name=f"pu{t}") for t, (_, bw, _) in enumerate(bins)]
                for ec2 in range(EC2):
                    w3 = uw3[:, ec2]
                    e3 = enc_sb[ec2][:].rearrange("p (two x) -> p two x", two=2)
                    for t, (boff, bw, _) in enumerate(bins):
                        nc.tensor.matmul(
                            psb[t][:],
                            lhsT=w3,
                            rhs=e3[:, :, boff:boff + bw],
                            start=(ec2 == 0),
                            stop=(ec2 == EC2 - 1),
                            perf_mode=DR,
                        )
                # tanh (+ per-position Wh bias), then v-weight into acc
                vcol = vt_sb[:, hc:hc + 1]
                for t, (boff, bw, segs) in enumerate(bins):
                    th = thp.tile([128, bw], BF, tag="th")
                    for pi, lo, w in segs:
                        # hidT columns are stored in placement order, so the
                        # Wh bias column for this segment is pi itself
                        nc.scalar.activation(
                            th[:, lo:lo + w], psb[t][:, lo:lo + w], AF.Tanh,
                            bias=bias_sb[:, hc * BL + pi:hc * BL + pi + 1],
                            scale=1.0 / U_SCALE,
                        )
                    if hc == 0:
                        nc.vector.tensor_tensor(
                            acc[:, boff:boff + bw], th[:],
                            vcol.to_broadcast([128, bw]), mybir.AluOpType.mult)
                    else:
                        tmp = tmpp.tile([128, bw], BF, tag="tmp")
                        nc.vector.tensor_tensor(
                            tmp[:], th[:], vcol.to_broadcast([128, bw]),
                            mybir.AluOpType.mult)
                        # last h-chunk: write the sum as bf16 for the
                        # partition-reduce matmul (single final rounding)
                        dst = accb if hc == HC - 1 else acc
                        nc.vector.tensor_add(
                            dst[:, boff:boff + bw], acc[:, boff:boff + bw], tmp[:])
                        if hc == HC - 1:
                            # fold the -1e10 mask row into partition 0 (both
                            # DVE inputs must share a base partition) so the
                            # ones-reduce needs no separate mask matmul
                            nc.vector.tensor_add(
                                accb[0:1, boff:boff + bw],
                                accb[0:1, boff:boff + bw],
                                am_sb[0:1, boff:boff + bw])

            # ---- partition reduce + softmax -----------------------------
            pe_g = [pep.tile([128, 512], F32, tag=f"pe{g}", name=f"pe{g}") for g in range(NG)]
            for g in range(NG):
                nc.vector.memset(pe_g[g][:], 0.0)
            for pi in range(NPOS):
                off, w = pl_off[pi]
                g, strip = pi // 4, 32 * (pi % 4)
                nc.tensor.matmul(
                    pe_g[g][strip:strip + 1, 0:w], lhsT=ones_c[:, 0:1],
                    rhs=accb[:, off:off + w],
                    start=True, stop=True, tile_position=(0, strip),
                )
            for g in range(NG):
                # one bank-wide exp covers all 4 strips; non-strip rows are
                # zero (memset) so they contribute harmless exp(0) garbage
                nc.scalar.activation(
                    res_g[g][:], pe_g[g][:], AF.Exp,
                    accum_out=ss_g[g][:],
                )
                # pad columns [w, 512) of each strip are exactly 0 in PSUM,
                # contributing exp(0)=1 each; subtract that known constant
                nc.vector.tensor_tensor(
                    ss_g[g][:], ss_g[g][:], corr_sb[:, g:g + 1],
                    mybir.AluOpType.subtract)
                nc.vector.reciprocal(rcp_g[g][:], ss_g[g][:])
                nc.vector.tensor_tensor(
                    res_g[g][:], res_g[g][:],
                    rcp_g[g][:, 0:1].to_broadcast([128, 512]),
                    mybir.AluOpType.mult,
                )
                rows = min(4, NPOS - 4 * g)
                src4 = res_g[g][:].rearrange("(f r) x -> f r x", f=4)[0:rows, 0, :]
                eng = (nc.sync, nc.scalar)[g % 2]
                eng.dma_start(out_d[4 * g:4 * g + rows, :], src4)

    nc.finalize()
    return nc


def _prep_inputs(hidden, encoder_outputs, mask, W_w, W_b, U_w, U_b, v_w):
    counts = (~mask).sum(axis=1)
    order = np.argsort(counts, kind="stable")
    # position p on core c holds row order[NCORES*p + c]
    rows = order.reshape(BL, NCORES)
    widths = [max(16, int(counts[rows[p]].max())) for p in range(BL)]
    widths = tuple(widths)
    assert all(w <= 512 for w in widths)

    placements, bins = _plan(widths)
    X = sum(widths)
    XP = -(-X // 16) * 16
    pl_off = [None] * BL
    for boff, bw, segs in bins:
        for pi, lo, w in segs:
            pl_off[pi] = (boff + lo, w)
    # original position -> placement idx
    pos2pl = [None] * BL
    for pi, p in enumerate(placements):
        pos2pl[p] = pi

    # ---- replicated weights ----
    U8 = np.ascontiguousarray((U_w * U_SCALE).T).astype(fp8)       # [E, H]
    uw8_np = np.ascontiguousarray(
        U8.reshape(EC2, 2, 128, HC, 128).transpose(3, 2, 0, 1, 4)
    ).reshape(HC, 128, EC2 * 2 * 128)
    wwT_np = np.ascontiguousarray(W_w.T).astype(bf16)
    wwT_np = np.ascontiguousarray(
        wwT_np.reshape(KC, 128, HC, 128).transpose(2, 1, 0, 3)
    ).reshape(HC, 128, KC * 128)
    vt_np = np.ascontiguousarray(v_w[0].reshape(HC, 128).T).astype(bf16)
    bc_np = np.ascontiguousarray((W_b + U_b).reshape(HC, 128).T).astype(np.float32)

    enc8_full = encoder_outputs.astype(fp8)                        # [S, B, E]

    NG = (BL + 3) // 4
    corr_np = np.zeros((128, NG), np.float32)
    for pi in range(BL):
        _, w = pl_off[pi]
        corr_np[32 * (pi % 4), pi // 4] = 512 - w

    idx_all = [np.nonzero(~mask[i])[0] for i in range(B)]
    in_maps = []
    for c in range(NCORES):
        enc_p = np.zeros((EC2, 128, 2, XP), fp8)
        am_p = np.full((XP,), NEG, np.float32)
        hid_rows = np.empty((BL, H), np.float32)
        for p in range(BL):
            r = int(rows[p, c])
            pi = pos2pl[p]
            off, w = pl_off[pi]
            ix = idx_all[r]
            cnt = len(ix)
            if cnt:
                # [cnt, E] -> [EC2, 2, 128, cnt]
                a = enc8_full[ix, r, :].T.reshape(EC2, 2, 128, cnt)
                enc_p[:, :, :, off:off + cnt] = a.transpose(0, 2, 1, 3)
                am_p[off:off + cnt] = 0.0
            hid_rows[pi] = hidden[r]
        hidT_c = np.ascontiguousarray(
            hid_rows.T.astype(bf16).reshape(KC, 128, BL).transpose(1, 0, 2)
        ).reshape(128, KC * BL)
        in_maps.append({
            "enc8": enc_p.reshape(EC2, 128, 2 * XP),
            "uw8": uw8_np,
            "wwT": wwT_np,
            "hidT": hidT_c,
            "vt": vt_np,
            "bc": bc_np,
            "amask": am_p.astype(bf16).reshape(1, XP),
            "corr": corr_np,
        })
    return in_maps, widths, rows, pos2pl, idx_all


def _run(in_maps, widths, trace=False):
    from concourse import bass_utils
    if widths not in _CACHE:
        _CACHE[widths] = _build_nc(widths)
    nc = _CACHE[widths]
    return bass_utils.run_bass_kernel_spmd(
        nc, in_maps, core_ids=list(range(NCORES)), trace=trace
    )


def kernel(hidden, encoder_outputs, mask, W_w, W_b, U_w, U_b, v_w,
           _trace=False, _return_bkr=False):
    hidden = np.asarray(hidden, dtype=np.float32)
    encoder_outputs = np.asarray(encoder_outputs, dtype=np.float32)
    mask = np.asarray(mask).astype(bool)
    W_w = np.asarray(W_w, dtype=np.float32)
    W_b = np.asarray(W_b, dtype=np.float32)
    U_w = np.asarray(U_w, dtype=np.float32)
    U_b = np.asarray(U_b, dtype=np.float32)
    v_w = np.asarray(v_w, dtype=np.float32)

    in_maps, widths, rows, pos2pl, idx_all = _prep_inputs(
        hidden, encoder_outputs, mask, W_w, W_b, U_w, U_b, v_w)
    bkr = _run(in_maps, widths, trace=_trace)

    out = np.zeros((B, S), np.float32)
    for c in range(NCORES):
        dev = bkr.results[c]["out"]                  # [BL, 512]
        for p in range(BL):
            r = int(rows[p, c])
            ix = idx_all[r]
            cnt = len(ix)
            if cnt:
                out[r, ix] = dev[pos2pl[p], :cnt]
            else:
                out[r, :] = np.float32(1.0 / S)
    if _return_bkr:
        return out, bkr
    return out
